# revision 1
# baseline (speedup 1.0000x reference)
"""ExpertPreferredRouter on 8 TRN2 NeuronCores.

Structure:
  - Host: transpose x shards (batch b = core%4, half h = core//4) to [D, H].
  - Phase A (device): logitsT = W @ x_half.T via PE (fp32), softmax over the
    expert (partition) axis -> probsT [64, 2048] per core.
  - Init: per-half top-64 extraction (max/match_replace), pair AllGather of
    (probsT half, candidates) -> full rows r [64, 4096] + merged init
    threshold t0 = exact 64th largest per row.
  - Phase C: damped-rank Jacobi waves on thresholds t_j: per wave, PE applies
    the cross-expert steal mask (strict-upper-triangular matmul on the
    selection mask), fused compare+accumulate gives per-row counts, and the
    threshold descends by up to 16 ranks via DVE max8 candidates.
    Fixpoint == exact greedy expert-preferred assignment.
  - Phase D: disjoint final selection mask -> M (priority matmul) and
    M_probs (masked prob sum matmul).
"""
import os
import sys
import types

import numpy as np

B, N, D, E = 4, 4096, 4096, 64
H = N // 2            # tokens per core (half a batch)
NCORES = 8
WAVES = 22            # numpy raw-rule convergence: 18-19 (dmax=8); margin
DMAX = 8
BIGSEL = float(2.0 ** 100)

TRACE = False         # set True (e.g. by test.py) to capture NTFF timing
LAST_EXEC_NS = None

_cache = {}


def _install_ntff_hook():
    if "antenv.axon_hooks" in sys.modules:
        return
    mod = types.ModuleType("antenv.axon_hooks")
    state = {"hook": None}
    mod.set_axon_ntff_profile_hook = lambda h: state.__setitem__("hook", h)
    mod.get_axon_ntff_profile_hook = lambda: state["hook"]
    sys.modules["antenv.axon_hooks"] = mod
    try:
        import antenv
        antenv.axon_hooks = mod
    except ImportError:
        pass
    try:
        from trn_agent_boot.trn_boot import _ntff_profile_via_ctypes
        mod.set_axon_ntff_profile_hook(
            _ntff_profile_via_ctypes("/opt/axon/libaxon_pjrt.so")
        )
    except Exception:
        pass


def _build_program():
    import concourse.bacc as bacc
    import concourse.mybir as mybir
    from concourse.tile import TileContext
    from concourse.masks import make_identity

    f32 = mybir.dt.float32
    bf16 = mybir.dt.bfloat16
    i32 = mybir.dt.int32
    Alu = mybir.AluOpType

    nc = bacc.Bacc("TRN2", target_bir_lowering=False, num_devices=NCORES)

    xt = nc.dram_tensor("xt", [D, H], f32, kind="ExternalInput")
    wt = nc.dram_tensor("wt", [D, E], f32, kind="ExternalInput")
    mo = nc.dram_tensor("mo", [1, N], f32, kind="ExternalOutput")
    po = nc.dram_tensor("po", [1, N], f32, kind="ExternalOutput")
    co = nc.dram_tensor("co", [E, 1], f32, kind="ExternalOutput")
    DEBUG = bool(int(os.environ.get("KDEBUG", "0")))
    if DEBUG:
        ro = nc.dram_tensor("ro", [E, N], f32, kind="ExternalOutput")
        t0o = nc.dram_tensor("t0o", [E, 1], f32, kind="ExternalOutput")
        cno = nc.dram_tensor("cno", [E, 128], f32, kind="ExternalOutput")

    with TileContext(nc) as tc:
        with (
            tc.tile_pool(name="persist", bufs=1) as pp,
            tc.tile_pool(name="work", bufs=1) as wp,
            tc.tile_pool(name="stream", bufs=3) as sp,
            tc.tile_pool(name="small", bufs=2) as smp,
            tc.tile_pool(name="dram", bufs=1, space="DRAM") as dp,
        ):
            # ---------------- Phase A: matmul ----------------
            # wt_sb[p, dc*64+e] = wt[dc*128+p, e]
            wt_sb = pp.tile([128, 32 * E], f32, tag="wt")
            nc.sync.dma_start(
                wt_sb[:].rearrange("p (c e) -> p c e", e=E),
                wt[:].rearrange("(c p) e -> p c e", p=128),
            )

            probsT = pp.tile([E, H], f32, tag="probsT")
            with tc.tile_pool(name="plog", bufs=1, space="PSUM") as plog_pool:
                psumL = plog_pool.tile([E, H], f32, tag="plog")
                for dc in range(32):
                    xchunk = sp.tile([128, H], f32, tag="xchunk")
                    nc.sync.dma_start(xchunk[:], xt[dc * 128 : (dc + 1) * 128, :])
                    for nt in range(4):
                        sl = slice(nt * 512, (nt + 1) * 512)
                        nc.tensor.matmul(
                            psumL[:, sl],
                            wt_sb[:, dc * E : (dc + 1) * E],
                            xchunk[:, sl],
                            start=(dc == 0),
                            stop=(dc == 31),
                        )
                # softmax over experts (partition axis), no max-subtraction
                # (|logits| <~ 5 so exp is safe in fp32)
                expT = wp.tile([E, H], f32, tag="expT")
                nc.scalar.activation(
                    expT[:], psumL[:], mybir.ActivationFunctionType.Exp
                )

            ones64 = pp.tile([E, 1], f32, tag="ones64")
            nc.vector.memset(ones64[:], 1.0)
            with tc.tile_pool(name="pz", bufs=1, space="PSUM") as pz_pool:
                pz = pz_pool.tile([1, H], f32, tag="pz")
                for ch in range(4):
                    sl = slice(ch * 512, (ch + 1) * 512)
                    nc.tensor.matmul(
                        pz[:, sl], ones64[:], expT[:, sl], start=True, stop=True
                    )
                zrow = wp.tile([1, H], f32, tag="zrow")
                nc.vector.reciprocal(zrow[:], pz[:])

            one1 = pp.tile([1, E], f32, tag="one1")
            nc.vector.memset(one1[:], 1.0)
            with tc.tile_pool(name="pw", bufs=1, space="PSUM") as pw_pool:
                pwb = pw_pool.tile([E, H], f32, tag="pwb")
                for ch in range(4):
                    sl = slice(ch * 512, (ch + 1) * 512)
                    nc.tensor.matmul(
                        pwb[:, sl], one1[:], zrow[:, sl], start=True, stop=True
                    )
                nc.vector.tensor_mul(probsT[:], expT[:], pwb[:])

            # ---------------- Init: per-half top-64 ----------------
            candL = wp.tile([E, 64], f32, tag="candL")
            wrkA = wp.tile([E, H], f32, tag="wrkA")
            wrkB = wp.tile([E, H], f32, tag="wrkB")
            nc.vector.tensor_copy(wrkA[:], probsT[:])
            cur, nxt = wrkA, wrkB
            for rnd in range(8):
                m8 = smp.tile([E, 8], f32, tag="m8")
                nc.vector.max(m8[:], cur[:])
                nc.vector.tensor_copy(candL[:, rnd * 8 : rnd * 8 + 8], m8[:])
                if rnd < 7:
                    nc.vector.match_replace(
                        out=nxt[:], in_to_replace=m8[:], in_values=cur[:],
                        imm_value=-1e38,
                    )
                    cur, nxt = nxt, cur

            # ---------------- AllGather pair {c, c+4} ----------------
            agin = dp.tile([E, H + 64], f32)
            agout = dp.tile([2, E, H + 64], f32)
            nc.sync.dma_start(agin[:, :H], probsT[:])
            nc.sync.dma_start(agin[:, H:], candL[:])
            nc.gpsimd.collective_compute(
                "AllGather",
                mybir.AluOpType.bypass,
                replica_groups=[[0, 4], [1, 5], [2, 6], [3, 7]],
                ins=[agin.opt()],
                outs=[agout.opt()],
            )
            r_sb = pp.tile([E, N], f32, tag="r")
            candAB = wp.tile([E, 128], f32, tag="candAB")
            for h in range(2):
                nc.sync.dma_start(r_sb[:, h * H : (h + 1) * H], agout[h, :, :H])
                nc.sync.dma_start(candAB[:, h * 64 : (h + 1) * 64], agout[h, :, H:])

            # ---------------- t0 = 64th largest of merged halves ----------------
            # union-kth: t0 = max_{i+j=64} min(A_i, B_j), A_0 = B_0 = +inf
            apad = wp.tile([E, 65], f32, tag="apad")
            brev = wp.tile([E, 65], f32, tag="brev")
            nc.vector.memset(apad[:, :1], 1e38)
            nc.vector.tensor_copy(apad[:, 1:], candAB[:, :64])
            nc.vector.memset(brev[:, 64:], 1e38)
            # brev[:, k] = B_(64-k) = candB reversed (negative-step read AP)
            nc.vector.tensor_copy(brev[:, :64], candAB[:, 64:][:, ::-1])
            tmin = wp.tile([E, 65], f32, tag="tmin")
            nc.vector.tensor_tensor(
                tmin[:], apad[:], brev[:], op=Alu.min
            )
            t_vec = pp.tile([E, 1], f32, tag="t")
            nc.vector.tensor_reduce(
                t_vec[:], tmin[:], axis=mybir.AxisListType.X, op=Alu.max
            )

            if DEBUG:
                nc.sync.dma_start(ro[:], r_sb[:])
                nc.sync.dma_start(t0o[:], t_vec[:])
                nc.sync.dma_start(cno[:], candAB[:])

            # ---------------- wave constants ----------------
            ident64 = pp.tile([E, E], f32, tag="ident64")
            make_identity(nc, ident64)
            # ustrict[j', j] = 1 if j' > j (bf16 weights for the msk matmul)
            iota_col = pp.tile([E, E], i32, tag="iotacol")
            nc.gpsimd.iota(iota_col[:], pattern=[[1, E]], base=0, channel_multiplier=0)
            ustrict = pp.tile([E, E], bf16, tag="ustrict")
            # row p: 1 where col < p  <=>  iota_col[p, c] = c < p
            pidx = pp.tile([E, 1], i32, tag="pidx")
            nc.gpsimd.iota(pidx[:], pattern=[[0, 1]], base=0, channel_multiplier=1)
            pidx_f = pp.tile([E, 1], f32, tag="pidxf")
            nc.vector.tensor_copy(pidx_f[:], pidx[:])
            iota_f = pp.tile([E, E], f32, tag="iotaf")
            nc.vector.tensor_copy(iota_f[:], iota_col[:])
            nc.vector.tensor_scalar(
                ustrict[:], iota_f[:], pidx_f[:], -BIGSEL, op0=Alu.is_lt, op1=Alu.mult
            )
            iota16 = pp.tile([E, 16], f32, tag="iota16")
            i16 = pp.tile([E, 16], i32, tag="i16")
            nc.gpsimd.iota(i16[:], pattern=[[1, 16]], base=0, channel_multiplier=0)
            nc.vector.tensor_copy(iota16[:], i16[:])
            jvec_bf = pp.tile([E, 1], bf16, tag="jvecbf")
            nc.vector.tensor_copy(jvec_bf[:], pidx_f[:])
            ones_bf = pp.tile([E, 1], bf16, tag="onesbf")
            nc.vector.memset(ones_bf[:], 1.0)

            msk = pp.tile([E, N], bf16, tag="msk1", name="msk_init")
            nc.vector.memset(msk[:], 0.0)
            w_sb = wp.tile([E, N], f32, tag="wsb")
            w2_sb = wp.tile([E, N], f32, tag="w2sb")
            cand16 = smp.tile([E, 16], f32, tag="cand16")
            cntp = smp.tile([E, 8], f32, tag="cntp")

            NCH = 8
            CW = N // NCH

            # ---------------- Phase C: waves ----------------
            # msk fed to the steal matmul uses the RAW rule (r >= t): steal
            # signals propagate in one hop, converging ~2x faster than the
            # masked rule. Counts/candidates use the masked values (pm).
            sel_m = wp.tile([E, N], bf16, tag="selm")
            for wv in range(WAVES + 1):
                last = wv == WAVES
                cand8 = smp.tile([E, 8], f32, tag="cand8")
                cntp = smp.tile([E, 8], f32, tag="cntp")
                if not last:
                    # raw-rule mask for next wave's steal matmul
                    mskn = pp.tile([E, N], bf16, tag=f"msk{wv % 2}", name=f"mskn{wv}")
                    nc.vector.tensor_scalar(
                        mskn[:], r_sb[:], t_vec[:], None, op0=Alu.is_ge
                    )
                else:
                    msk01 = pp.tile([E, N], bf16, tag="msk01")
                with tc.tile_pool(name=f"pmw{wv}", bufs=4, space="PSUM") as pmp:
                    for ch in range(NCH):
                        sl = slice(ch * CW, (ch + 1) * CW)
                        pm = pmp.tile([E, CW], f32, tag="pm")
                        nc.tensor.matmul(
                            pm[:], ident64[:], r_sb[:, sl], start=True, stop=False
                        )
                        nc.tensor.matmul(
                            pm[:], ustrict[:], msk[:, sl], start=False, stop=True
                        )
                        dst = msk01 if last else sel_m
                        nc.vector.tensor_scalar(
                            dst[:, sl], pm[:], t_vec[:], None,
                            op0=Alu.is_ge, op1=Alu.add,
                            accum_out=cntp[:, ch : ch + 1],
                        )
                        if not last:
                            # w = masked row with >=t values knocked out
                            nc.vector.scalar_tensor_tensor(
                                w_sb[:, sl], sel_m[:, sl], -BIGSEL, pm[:],
                                op0=Alu.mult, op1=Alu.add,
                            )
                if last:
                    cnt = smp.tile([E, 1], f32, tag="cnt")
                    nc.vector.tensor_reduce(
                        cnt[:], cntp[:], axis=mybir.AxisListType.X, op=Alu.add
                    )
                    nc.sync.dma_start(co[:], cnt[:])
                    msk = msk01
                    break

                cnt = smp.tile([E, 1], f32, tag="cnt")
                nc.vector.tensor_reduce(
                    cnt[:], cntp[:], axis=mybir.AxisListType.X, op=Alu.add
                )
                # d = clamp(64 - cnt, 0, DMAX); dm1 = d - 1
                dm1 = smp.tile([E, 1], f32, tag="dm1")
                nc.vector.tensor_scalar(
                    dm1[:], cnt[:], -1.0, 64.0,
                    op0=Alu.mult, op1=Alu.add,
                )
                nc.vector.tensor_scalar_min(dm1[:], dm1[:], float(DMAX))
                # dm1 = d - 1 (=-1 when d=0 -> onehot all zero)
                nc.vector.tensor_scalar_add(dm1[:], dm1[:], -1.0)

                # candidates: top-8 of w rows
                nc.vector.max(cand8[:], w_sb[:])

                # t_new = cand8[d-1]  (keep t when d == 0)
                oh = smp.tile([E, 8], f32, tag="oh")
                nc.vector.tensor_scalar(
                    oh[:], iota16[:, :8], dm1[:], None, op0=Alu.is_equal
                )
                tsel = smp.tile([E, 8], f32, tag="tsel")
                nc.vector.tensor_mul(tsel[:], cand8[:], oh[:])
                tnew = smp.tile([E, 1], f32, tag="tnew")
                nc.vector.tensor_reduce(
                    tnew[:], tsel[:], axis=mybir.AxisListType.X, op=Alu.add
                )
                z = smp.tile([E, 1], f32, tag="z")
                nc.vector.tensor_scalar(
                    z[:], dm1[:], -1.0, None, op0=Alu.is_equal
                )
                zk = smp.tile([E, 1], f32, tag="zk")
                nc.vector.tensor_mul(zk[:], z[:], t_vec[:])
                t_vec = pp.tile([E, 1], f32, tag=f"t{wv % 2}", name=f"tvec{wv}")
                nc.vector.tensor_add(t_vec[:], tnew[:], zk[:])
                msk = mskn

            # ---------------- Phase D: outputs ----------------
            # M = sum_j j * msk01[j, n]  (disjoint selection)
            psel = wp.tile([E, N], f32, tag="psel")
            nc.vector.tensor_mul(psel[:], r_sb[:], msk[:])
            mo_sb = wp.tile([1, N], f32, tag="mo")
            po_sb = wp.tile([1, N], f32, tag="po")
            with tc.tile_pool(name="pout", bufs=4, space="PSUM") as pop:
                for ch in range(8):
                    sl = slice(ch * 512, (ch + 1) * 512)
                    pmm = pop.tile([1, 512], f32, tag="pmm")
                    nc.tensor.matmul(
                        pmm[:], jvec_bf[:], msk[:, sl], start=True, stop=True
                    )
                    nc.vector.tensor_copy(mo_sb[:, sl], pmm[:])
                    ppp = pop.tile([1, 512], f32, tag="ppp")
                    nc.tensor.matmul(
                        ppp[:], ones64[:], psel[:, sl], start=True, stop=True
                    )
                    nc.vector.tensor_copy(po_sb[:, sl], ppp[:])
            nc.sync.dma_start(mo[:], mo_sb[:])
            nc.sync.dma_start(po[:], po_sb[:])

    nc.compile()
    return nc


def kernel(x, W, c):
    global LAST_EXEC_NS
    from concourse import bass_utils

    x = np.asarray(x, dtype=np.float32)
    W = np.asarray(W, dtype=np.float32)

    if "nc" not in _cache:
        _cache["nc"] = _build_program()
    nc = _cache["nc"]

    wt_host = np.ascontiguousarray(W.T)  # [D, E]
    in_maps = []
    for core in range(NCORES):
        b, h = core % B, core // B
        xt_host = np.ascontiguousarray(x[b, h * H : (h + 1) * H, :].T)  # [D, H]
        in_maps.append({"xt": xt_host, "wt": wt_host})

    trace = TRACE
    if trace:
        _install_ntff_hook()
    res = bass_utils.run_bass_kernel_spmd(
        nc, in_maps, core_ids=list(range(NCORES)), trace=trace
    )
    LAST_EXEC_NS = res.exec_time_ns

    M = np.zeros((B, N), dtype=np.int32)
    P = np.zeros((B, N), dtype=np.float32)
    for b in range(B):
        out = res.results[b]
        cnt = out["co"][:, 0]
        if not np.allclose(cnt, 64.0):
            print(f"[kernel] WARNING: batch {b} expert counts != 64: "
                  f"min={cnt.min()} max={cnt.max()}", file=sys.stderr)
        M[b] = np.rint(out["mo"][0]).astype(np.int32)
        P[b] = out["po"][0].astype(np.float32)
    return M, P



# revision 6
# speedup vs baseline: 1.6913x; 1.6913x over previous
"""ExpertPreferredRouter on 8 TRN2 NeuronCores — folded-128 log-space version.

Structure (per core; batch b = core%4, half h = core//4, 2048 tokens/core):
  - Fold: local token lt -> partition group u = lt//1024, col = lt%1024.
    All wave tensors are [128, *] (expert j + 64u on partitions) so DVE and PE
    run at full partition width.  The algorithm is value-based and therefore
    column/partition-permutation invariant; each core's view of the full 4096
    tokens is [own fold | partner fold] in AllGather replica order.
  - Phase A: logits into one [128, 1024] PSUM via dual weight blocks
    ([wt|0] for u=0 tokens, [0|wt] for u=1); log-softmax s = logits - ln(Z)
    (Ln on the scalar engine; no single-partition DVE reciprocal).
  - Init: 8 max8/match_replace rounds -> per-(j,u) top-64; AllGather of the
    s-tile overlapped under init; second small AllGather of candidates;
    merged [64,256] pool -> exact t0 = global 64th per expert.
  - Waves: steal mask via block-diag strict-upper matmul on [128, *] tiles,
    fused compare+count, per-chunk max8 candidates merged to a [64,32] pool,
    threshold descends up to DMAX=16 ranks/wave (cand17 = [t, top16(pool)]).
  - Final masked pass -> M (priority matmul) and P = exp(selected s).
"""
import os
import sys
import types

import numpy as np

B, N, D, E = 4, 4096, 4096, 64
H = N // 2            # tokens per core (half a batch)
NF = H                # folded full-row width: [128, NF] covers all N tokens
NCORES = 8
WAVES = 14            # content waves; sim (dmax=16 pool) exact at 11; margin 3
DMAX = 16
BIGSEL = float(2.0 ** 100)

TRACE = False         # set True (e.g. by test.py) to capture NTFF timing
LAST_EXEC_NS = None

_cache = {}

# token unfold maps (host side): token n -> (partition group u, column)
_n = np.arange(N)
_u_of_n = (_n // 1024) % 2
_col_of_n = _n % 1024 + 1024 * (_n // 2048)


def _install_ntff_hook():
    if "antenv.axon_hooks" in sys.modules:
        return
    mod = types.ModuleType("antenv.axon_hooks")
    state = {"hook": None}
    mod.set_axon_ntff_profile_hook = lambda h: state.__setitem__("hook", h)
    mod.get_axon_ntff_profile_hook = lambda: state["hook"]
    sys.modules["antenv.axon_hooks"] = mod
    try:
        import antenv
        antenv.axon_hooks = mod
    except ImportError:
        pass
    try:
        from trn_agent_boot.trn_boot import _ntff_profile_via_ctypes
        mod.set_axon_ntff_profile_hook(
            _ntff_profile_via_ctypes("/opt/axon/libaxon_pjrt.so")
        )
    except Exception:
        pass


def _build_program():
    import concourse.bacc as bacc
    import concourse.mybir as mybir
    from concourse.tile import TileContext
    from concourse.masks import make_identity

    f32 = mybir.dt.float32
    bf16 = mybir.dt.bfloat16
    i32 = mybir.dt.int32
    Alu = mybir.AluOpType
    Act = mybir.ActivationFunctionType

    nc = bacc.Bacc("TRN2", target_bir_lowering=False, num_devices=NCORES)

    xt = nc.dram_tensor("xt", [D, H], f32, kind="ExternalInput")
    wt = nc.dram_tensor("wt", [D, E], f32, kind="ExternalInput")
    mo = nc.dram_tensor("mo", [2, NF], f32, kind="ExternalOutput")
    po = nc.dram_tensor("po", [2, NF], f32, kind="ExternalOutput")
    co = nc.dram_tensor("co", [E, 1], f32, kind="ExternalOutput")
    DEBUG = bool(int(os.environ.get("KDEBUG", "0")))
    if DEBUG:
        ro = nc.dram_tensor("ro", [128, NF], f32, kind="ExternalOutput")
        t0o = nc.dram_tensor("t0o", [E, 1], f32, kind="ExternalOutput")

    with TileContext(nc) as tc:
        with (
            tc.tile_pool(name="persist", bufs=1) as pp,
            tc.tile_pool(name="work", bufs=1) as wp,
            tc.tile_pool(name="stream", bufs=3) as sp,
            tc.tile_pool(name="small", bufs=2) as smp,
            tc.tile_pool(name="dram", bufs=1, space="DRAM") as dp,
        ):
            # ---------------- constants (issued early; overlap phase A DMA) ----
            pi = pp.tile([128, 1], i32, tag="pi")
            nc.gpsimd.iota(pi[:], pattern=[[0, 1]], base=0, channel_multiplier=1)
            pif = pp.tile([128, 1], f32, tag="pif")
            nc.vector.tensor_copy(pif[:], pi[:])
            hp = pp.tile([128, 1], f32, tag="hp")
            nc.vector.tensor_scalar(hp[:], pif[:], 64.0, None, op0=Alu.is_ge)
            jp = pp.tile([128, 1], f32, tag="jp")
            nc.vector.scalar_tensor_tensor(
                jp[:], hp[:], -64.0, pif[:], op0=Alu.mult, op1=Alu.add
            )
            ci = pp.tile([128, 128], i32, tag="ci")
            nc.gpsimd.iota(ci[:], pattern=[[1, 128]], base=0, channel_multiplier=0)
            cif = pp.tile([128, 128], f32, tag="cif")
            nc.vector.tensor_copy(cif[:], ci[:])
            hc = pp.tile([128, 128], f32, tag="hc")
            nc.vector.tensor_scalar(hc[:], cif[:], 64.0, None, op0=Alu.is_ge)
            jcol = pp.tile([128, 128], f32, tag="jcol")
            nc.vector.scalar_tensor_tensor(
                jcol[:], hc[:], -64.0, cif[:], op0=Alu.mult, op1=Alu.add
            )
            cond1 = wp.tile([128, 128], f32, tag="cond1")
            nc.vector.tensor_scalar(cond1[:], jcol[:], jp[:], None, op0=Alu.is_lt)
            cond2 = wp.tile([128, 128], f32, tag="cond2")
            nc.vector.tensor_scalar(cond2[:], hc[:], hp[:], None, op0=Alu.is_equal)
            # ustrict[p=j'+64h', c=j+64h] = -BIG if j' > j and h' == h
            ustrict = pp.tile([128, 128], bf16, tag="ustrict")
            nc.vector.scalar_tensor_tensor(
                ustrict[:], cond1[:], -BIGSEL, cond2[:], op0=Alu.mult, op1=Alu.mult
            )
            ident128 = pp.tile([128, 128], f32, tag="ident128")
            make_identity(nc, ident128)
            # Wz [128, 2]: col0 = 1-hp, col1 = hp (u-half fold for column sums)
            Wz = pp.tile([128, 2], f32, tag="Wz")
            nc.vector.tensor_scalar(
                Wz[:, 0:1], hp[:], -1.0, 1.0, op0=Alu.mult, op1=Alu.add
            )
            nc.vector.tensor_copy(Wz[:, 1:2], hp[:])
            # Wb [2, 128]: row p -> 1 where column's u-half == p (broadcast up)
            ci2 = pp.tile([2, 128], i32, tag="ci2")
            nc.gpsimd.iota(ci2[:], pattern=[[1, 128]], base=0, channel_multiplier=0)
            ci2f = pp.tile([2, 128], f32, tag="ci2f")
            nc.vector.tensor_copy(ci2f[:], ci2[:])
            hc2 = pp.tile([2, 128], f32, tag="hc2")
            nc.vector.tensor_scalar(hc2[:], ci2f[:], 64.0, None, op0=Alu.is_ge)
            pi2 = pp.tile([2, 1], i32, tag="pi2")
            nc.gpsimd.iota(pi2[:], pattern=[[0, 1]], base=0, channel_multiplier=1)
            pi2f = pp.tile([2, 1], f32, tag="pi2f")
            nc.vector.tensor_copy(pi2f[:], pi2[:])
            Wb = pp.tile([2, 128], f32, tag="Wb")
            nc.vector.tensor_scalar(Wb[:], hc2[:], pi2f[:], None, op0=Alu.is_equal)
            # Wfold [128, 64]: W[p, c] = (p%64 == c) (sums both u-halves)
            c64 = pp.tile([128, 64], i32, tag="c64")
            nc.gpsimd.iota(c64[:], pattern=[[1, 64]], base=0, channel_multiplier=0)
            c64f = pp.tile([128, 64], f32, tag="c64f")
            nc.vector.tensor_copy(c64f[:], c64[:])
            Wfold = pp.tile([128, 64], f32, tag="Wfold")
            nc.vector.tensor_scalar(Wfold[:], c64f[:], jp[:], None, op0=Alu.is_equal)
            # Wtop/Wlow [128, 64]: select u=0 / u=1 partition half
            Wtop = pp.tile([128, 64], f32, tag="Wtop")
            nc.vector.tensor_scalar(Wtop[:], Wfold[:], Wz[:, 0:1], None, op0=Alu.mult)
            Wlow = pp.tile([128, 64], f32, tag="Wlow")
            nc.vector.tensor_scalar(Wlow[:], Wfold[:], hp[:], None, op0=Alu.mult)
            # Wdup [64, 128]: W[p, c] = (c%64 == p) (broadcast t to both halves)
            pi64 = pp.tile([64, 1], i32, tag="pi64")
            nc.gpsimd.iota(pi64[:], pattern=[[0, 1]], base=0, channel_multiplier=1)
            pi64f = pp.tile([64, 1], f32, tag="pi64f")
            nc.vector.tensor_copy(pi64f[:], pi64[:])
            ci64 = pp.tile([64, 128], i32, tag="ci64")
            nc.gpsimd.iota(ci64[:], pattern=[[1, 128]], base=0, channel_multiplier=0)
            ci64f = pp.tile([64, 128], f32, tag="ci64f")
            nc.vector.tensor_copy(ci64f[:], ci64[:])
            hc64 = pp.tile([64, 128], f32, tag="hc64")
            nc.vector.tensor_scalar(hc64[:], ci64f[:], 64.0, None, op0=Alu.is_ge)
            jc64 = pp.tile([64, 128], f32, tag="jc64")
            nc.vector.scalar_tensor_tensor(
                jc64[:], hc64[:], -64.0, ci64f[:], op0=Alu.mult, op1=Alu.add
            )
            Wdup = pp.tile([64, 128], f32, tag="Wdup")
            nc.vector.tensor_scalar(Wdup[:], jc64[:], pi64f[:], None, op0=Alu.is_equal)
            # Wm [128, 2] bf16: col u = jp * (hp == u) (priority readout)
            Wm = pp.tile([128, 2], bf16, tag="Wm")
            nc.vector.tensor_tensor(Wm[:, 0:1], jp[:], Wz[:, 0:1], op=Alu.mult)
            nc.vector.tensor_tensor(Wm[:, 1:2], jp[:], hp[:], op=Alu.mult)
            # iota17 [64, 17] for cand indexing
            i17 = pp.tile([64, 17], i32, tag="i17")
            nc.gpsimd.iota(i17[:], pattern=[[1, 17]], base=0, channel_multiplier=0)
            iota17 = pp.tile([64, 17], f32, tag="iota17")
            nc.vector.tensor_copy(iota17[:], i17[:])

            # ---------------- Phase A: logits -> log-softmax ----------------
            # wtbig[p, dc, 0:64] = wt[dc*128+p, :]; [64:128] = 0; [128:192] = wt
            # WtLow(dc) = wtbig[:, dc, 0:128]  -> expert channels 0-63 (u=0)
            # WtHigh(dc) = wtbig[:, dc, 64:192] -> channels 64-127 (u=1)
            r2 = pp.tile([128, NF], f32, tag="r2")   # full folded s
            with tc.tile_pool(name="phA", bufs=1) as pa:
                wtbig = pa.tile([128, 32, 192], f32, tag="wtbig")
                nc.vector.memset(wtbig[:, :, 64:128], 0.0)
                nc.sync.dma_start(
                    wtbig[:, :, 0:64],
                    wt[:].rearrange("(c p) e -> p c e", p=128),
                )
                nc.sync.dma_start(
                    wtbig[:, :, 128:192],
                    wt[:].rearrange("(c p) e -> p c e", p=128),
                )
                with tc.tile_pool(name="plog", bufs=1, space="PSUM") as plog_pool:
                    psumA = plog_pool.tile([128, 1024], f32, tag="plog")
                    for dc in range(32):
                        xchunk = sp.tile([128, H], f32, tag="xchunk")
                        nc.sync.dma_start(xchunk[:], xt[dc * 128: (dc + 1) * 128, :])
                        for cg in range(2):
                            sl = slice(cg * 512, (cg + 1) * 512)
                            nc.tensor.matmul(
                                psumA[:, sl],
                                wtbig[:, dc, 0:128],
                                xchunk[:, sl],
                                start=(dc == 0), stop=False,
                            )
                            nc.tensor.matmul(
                                psumA[:, sl],
                                wtbig[:, dc, 64:192],
                                xchunk[:, 1024 + cg * 512: 1024 + (cg + 1) * 512],
                                start=False, stop=(dc == 31),
                            )
                    # Z per (u, col) by folding expert partitions; then ln
                    expT = wp.tile([128, 1024], f32, tag="expT")
                    nc.scalar.activation(expT[:], psumA[:], Act.Exp)
                    lnZ = wp.tile([2, 1024], f32, tag="lnZ")
                    with tc.tile_pool(name="pz", bufs=1, space="PSUM") as pz_pool:
                        pz = pz_pool.tile([2, 1024], f32, tag="pz")
                        for cg in range(2):
                            sl = slice(cg * 512, (cg + 1) * 512)
                            nc.tensor.matmul(
                                pz[:, sl], Wz[:], expT[:, sl], start=True, stop=True
                            )
                        nc.scalar.activation(lnZ[:], pz[:], Act.Ln)
                    with tc.tile_pool(name="pb", bufs=1, space="PSUM") as pb_pool:
                        lnZb = pb_pool.tile([128, 1024], f32, tag="lnZb")
                        for cg in range(2):
                            sl = slice(cg * 512, (cg + 1) * 512)
                            nc.tensor.matmul(
                                lnZb[:, sl], Wb[:], lnZ[:, sl], start=True, stop=True
                            )
                        # DVE can read only one PSUM input; stage lnZb via the
                        # (otherwise idle) scalar engine
                        lnZbs = wp.tile([128, 1024], f32, tag="lnZbs")
                        nc.scalar.activation(lnZbs[:], lnZb[:], Act.Copy)
                        # s (own fold) = logits - lnZ -> r2 cols 0:1024
                        nc.vector.tensor_tensor(
                            r2[:, 0:1024], psumA[:], lnZbs[:], op=Alu.subtract
                        )

            # ---------------- AllGather 1: s-tile (overlaps with init) -------
            agin1 = dp.tile([128, 1024], f32)
            agout1 = dp.tile([2, 128, 1024], f32)
            nc.sync.dma_start(agin1[:], r2[:, 0:1024])
            nc.gpsimd.collective_compute(
                "AllGather",
                mybir.AluOpType.bypass,
                replica_groups=[[0, 4], [1, 5], [2, 6], [3, 7]],
                ins=[agin1.opt()],
                outs=[agout1.opt()],
            )

            # ---------------- Init: per-(j,u) top-64 of own half -------------
            cand2 = wp.tile([128, 64], f32, tag="cand2")
            wrkA = wp.tile([128, 1024], f32, tag="wrkA")
            wrkB = wp.tile([128, 1024], f32, tag="wrkB")
            nc.vector.tensor_copy(wrkA[:], r2[:, 0:1024])
            cur, nxt = wrkA, wrkB
            for rnd in range(8):
                m8 = smp.tile([128, 8], f32, tag="m8")
                nc.vector.max(m8[:], cur[:])
                nc.vector.tensor_copy(cand2[:, rnd * 8: rnd * 8 + 8], m8[:])
                if rnd < 7:
                    nc.vector.match_replace(
                        out=nxt[:], in_to_replace=m8[:], in_values=cur[:],
                        imm_value=-1e38,
                    )
                    cur, nxt = nxt, cur

            # gathered s-tiles -> r2 full (replica order; own rewrite harmless)
            for g in range(2):
                nc.sync.dma_start(
                    r2[:, g * 1024: (g + 1) * 1024], agout1[g, :, :]
                )

            # ---------------- AllGather 2: candidates ----------------
            agin2 = dp.tile([128, 64], f32)
            agout2 = dp.tile([2, 128, 64], f32)
            nc.sync.dma_start(agin2[:], cand2[:])
            nc.gpsimd.collective_compute(
                "AllGather",
                mybir.AluOpType.bypass,
                replica_groups=[[0, 4], [1, 5], [2, 6], [3, 7]],
                ins=[agin2.opt()],
                outs=[agout2.opt()],
            )
            candAll = wp.tile([128, 128], f32, tag="candAll")
            for g in range(2):
                nc.sync.dma_start(
                    candAll[:, g * 64: (g + 1) * 64], agout2[g, :, :]
                )

            # fold 4 sorted-64 lists -> [64, 256] pool; 8 rounds -> exact 64th
            poolsb = wp.tile([64, 256], f32, tag="poolsb")
            with tc.tile_pool(name="pfold", bufs=1, space="PSUM") as pf_pool:
                poolp = pf_pool.tile([64, 256], f32, tag="poolp")
                nc.tensor.matmul(poolp[:, 0:64], Wtop[:], candAll[:, 0:64],
                                 start=True, stop=True)
                nc.tensor.matmul(poolp[:, 64:128], Wlow[:], candAll[:, 0:64],
                                 start=True, stop=True)
                nc.tensor.matmul(poolp[:, 128:192], Wtop[:], candAll[:, 64:128],
                                 start=True, stop=True)
                nc.tensor.matmul(poolp[:, 192:256], Wlow[:], candAll[:, 64:128],
                                 start=True, stop=True)
                nc.vector.tensor_copy(poolsb[:], poolp[:])
            poolwk = wp.tile([64, 256], f32, tag="poolwk")
            curp, nxtp = poolsb, poolwk
            t_vec = pp.tile([64, 1], f32, tag="t")
            for rnd in range(8):
                m8b = smp.tile([64, 8], f32, tag="m8b")
                nc.vector.max(m8b[:], curp[:])
                if rnd < 7:
                    nc.vector.match_replace(
                        out=nxtp[:], in_to_replace=m8b[:], in_values=curp[:],
                        imm_value=-1e38,
                    )
                    curp, nxtp = nxtp, curp
                else:
                    nc.vector.tensor_copy(t_vec[:], m8b[:, 7:8])

            if DEBUG:
                nc.sync.dma_start(ro[:], r2[:])
                nc.sync.dma_start(t0o[:], t_vec[:])

            # t broadcast to both u-halves
            t2sb = pp.tile([128, 1], f32, tag="t2sbi", name="t2_init")
            with tc.tile_pool(name="pt2i", bufs=1, space="PSUM") as pt_pool:
                t2p = pt_pool.tile([128, 1], f32, tag="t2p")
                nc.tensor.matmul(t2p[:], Wdup[:], t_vec[:], start=True, stop=True)
                nc.vector.tensor_copy(t2sb[:], t2p[:])

            # wave 0: raw claims mask only (steal matmul would be on zeros)
            msk = pp.tile([128, NF], bf16, tag="mskA", name="msk_init")
            nc.vector.tensor_scalar(msk[:], r2[:], t2sb[:], None, op0=Alu.is_ge)

            candp = smp.tile([128, 16], f32, tag="candp")
            NCH = 2
            CW = NF // NCH  # 1024

            # ---------------- waves ----------------
            for wv in range(1, WAVES + 2):
                last = wv == WAVES + 1
                cntp2 = smp.tile([128, NCH], f32, tag="cntp2")
                if last:
                    msk01 = pp.tile([128, NF], bf16, tag="msk01")
                with tc.tile_pool(name=f"pmw{wv}", bufs=2, space="PSUM") as pmp:
                    for ch in range(NCH):
                        pm = pmp.tile([128, CW], f32, tag="pm")
                        for cg in range(CW // 512):
                            sl = slice(ch * CW + cg * 512, ch * CW + (cg + 1) * 512)
                            psl = slice(cg * 512, (cg + 1) * 512)
                            nc.tensor.matmul(
                                pm[:, psl], ident128[:], r2[:, sl],
                                start=True, stop=False,
                            )
                            nc.tensor.matmul(
                                pm[:, psl], ustrict[:], msk[:, sl],
                                start=False, stop=True,
                            )
                        sl = slice(ch * CW, (ch + 1) * CW)
                        if last:
                            dsl = msk01[:, sl]
                        else:
                            selm = smp.tile([128, CW], bf16, tag="selm")
                            dsl = selm[:]
                        nc.vector.tensor_scalar(
                            dsl, pm[:], t2sb[:], None,
                            op0=Alu.is_ge, op1=Alu.add,
                            accum_out=cntp2[:, ch: ch + 1],
                        )
                        if not last:
                            wchunk = smp.tile([128, CW], f32, tag="wchunk")
                            nc.vector.scalar_tensor_tensor(
                                wchunk[:], dsl, -BIGSEL, pm[:],
                                op0=Alu.mult, op1=Alu.add,
                            )
                            nc.vector.max(candp[:, ch * 8: ch * 8 + 8], wchunk[:])

                # counts: reduce chunks, then fold the two u-halves
                cnt2 = smp.tile([128, 1], f32, tag="cnt2")
                nc.vector.tensor_reduce(
                    cnt2[:], cntp2[:], axis=mybir.AxisListType.X, op=Alu.add
                )
                with tc.tile_pool(name=f"pcw{wv}", bufs=1, space="PSUM") as pc_pool:
                    cntf = pc_pool.tile([64, 1], f32, tag="cntf")
                    nc.tensor.matmul(cntf[:], Wfold[:], cnt2[:], start=True, stop=True)
                    if last:
                        cnt_sb = smp.tile([64, 1], f32, tag="cntsb")
                        nc.vector.tensor_copy(cnt_sb[:], cntf[:])
                        nc.sync.dma_start(co[:], cnt_sb[:])
                        msk = msk01
                        break

                    # candidate pool [64, 32] -> top-16 into cand17[1:17]
                    cand17 = smp.tile([64, 17], f32, tag="cand17")
                    nc.vector.tensor_copy(cand17[:, 0:1], t_vec[:])
                    pool32 = smp.tile([64, 32], f32, tag="pool32")
                    with tc.tile_pool(name=f"pdw{wv}", bufs=1, space="PSUM") as pd_pool:
                        poolq = pd_pool.tile([64, 32], f32, tag="poolq")
                        nc.tensor.matmul(poolq[:, 0:16], Wtop[:], candp[:],
                                         start=True, stop=True)
                        nc.tensor.matmul(poolq[:, 16:32], Wlow[:], candp[:],
                                         start=True, stop=True)
                        nc.vector.tensor_copy(pool32[:], poolq[:])
                    nc.vector.max(cand17[:, 1:9], pool32[:])
                    pool32b = smp.tile([64, 32], f32, tag="pool32b")
                    nc.vector.match_replace(
                        out=pool32b[:], in_to_replace=cand17[:, 1:9],
                        in_values=pool32[:], imm_value=-1e38,
                    )
                    nc.vector.max(cand17[:, 9:17], pool32b[:])

                    # d = clamp(64 - cnt, 0, DMAX); t = cand17[d]
                    dmv = smp.tile([64, 1], f32, tag="dmv")
                    nc.vector.tensor_scalar(
                        dmv[:], cntf[:], -1.0, 64.0, op0=Alu.mult, op1=Alu.add
                    )
                    nc.vector.tensor_scalar_min(dmv[:], dmv[:], float(DMAX))
                    nc.vector.tensor_scalar_max(dmv[:], dmv[:], 0.0)
                    oh = smp.tile([64, 17], f32, tag="oh")
                    nc.vector.tensor_scalar(
                        oh[:], iota17[:], dmv[:], None, op0=Alu.is_equal
                    )
                    tsel = smp.tile([64, 17], f32, tag="tsel")
                    nc.vector.tensor_tensor(tsel[:], oh[:], cand17[:], op=Alu.mult)
                    t_vec = pp.tile([64, 1], f32, tag=f"t{wv % 2}", name=f"tvec{wv}")
                    nc.vector.tensor_reduce(
                        t_vec[:], tsel[:], axis=mybir.AxisListType.X, op=Alu.add
                    )
                    t2sb = pp.tile([128, 1], f32, tag=f"t2sb{wv % 2}",
                                   name=f"t2_{wv}")
                    t2p2 = pc_pool.tile([128, 1], f32, tag="t2p2")
                    nc.tensor.matmul(t2p2[:], Wdup[:], t_vec[:], start=True, stop=True)
                    nc.vector.tensor_copy(t2sb[:], t2p2[:])
                mskn = pp.tile([128, NF], bf16, tag=f"msk{wv % 2}", name=f"mskn{wv}")
                nc.vector.tensor_scalar(mskn[:], r2[:], t2sb[:], None, op0=Alu.is_ge)
                msk = mskn

            # ---------------- outputs ----------------
            psel = wp.tile([128, NF], f32, tag="psel")
            nc.vector.tensor_tensor(psel[:], r2[:], msk[:], op=Alu.mult)
            mo_sb = wp.tile([2, NF], f32, tag="mo")
            po_sb = wp.tile([2, NF], f32, tag="po")
            with tc.tile_pool(name="pout", bufs=4, space="PSUM") as pop:
                for ch in range(NF // 512):
                    sl = slice(ch * 512, (ch + 1) * 512)
                    pmm = pop.tile([2, 512], f32, tag="pmm")
                    nc.tensor.matmul(
                        pmm[:], Wm[:], msk[:, sl], start=True, stop=True
                    )
                    nc.vector.tensor_copy(mo_sb[:, sl], pmm[:])
                    ppp = pop.tile([2, 512], f32, tag="ppp")
                    nc.tensor.matmul(
                        ppp[:], Wz[:], psel[:, sl], start=True, stop=True
                    )
                    nc.scalar.activation(po_sb[:, sl], ppp[:], Act.Exp)
            nc.sync.dma_start(mo[:], mo_sb[:])
            nc.sync.dma_start(po[:], po_sb[:])

    nc.compile()
    return nc


def kernel(x, W, c):
    global LAST_EXEC_NS
    from concourse import bass_utils

    x = np.asarray(x, dtype=np.float32)
    W = np.asarray(W, dtype=np.float32)

    if "nc" not in _cache:
        _cache["nc"] = _build_program()
    nc = _cache["nc"]

    wt_host = np.ascontiguousarray(W.T)  # [D, E]
    in_maps = []
    for core in range(NCORES):
        b, h = core % B, core // B
        xt_host = np.ascontiguousarray(x[b, h * H: (h + 1) * H, :].T)  # [D, H]
        in_maps.append({"xt": xt_host, "wt": wt_host})

    trace = TRACE
    if trace:
        _install_ntff_hook()
    res = bass_utils.run_bass_kernel_spmd(
        nc, in_maps, core_ids=list(range(NCORES)), trace=trace
    )
    LAST_EXEC_NS = res.exec_time_ns

    M = np.zeros((B, N), dtype=np.int32)
    P = np.zeros((B, N), dtype=np.float32)
    for b in range(B):
        out = res.results[b]
        cnt = out["co"][:, 0]
        if not np.allclose(cnt, 64.0):
            print(f"[kernel] WARNING: batch {b} expert counts != 64: "
                  f"min={cnt.min()} max={cnt.max()}", file=sys.stderr)
        # core b has h=0: cols 0:1024 = tokens 0:2048 folded (u = lt//1024,
        # col = lt%1024); cols 1024:2048 = tokens 2048:4096 folded.
        m2 = out["mo"]  # [2, 2048]
        p2 = out["po"]
        M[b, :] = np.rint(m2[_u_of_n, _col_of_n]).astype(np.int32)
        P[b, :] = p2[_u_of_n, _col_of_n].astype(np.float32)
    return M, P
